# revision 35
# baseline (speedup 1.0000x reference)
"""YOLO loss kernel for Trainium2 (Bass/Tile), data-parallel over 8 NeuronCores.

Math (per sample n, cell s; S=14, SS=196, B=2, C=20, D=30):
  t4 = target conf channel (binary 0/1), obj = t4, noobj = 1 - t4
  Per box b: overlap per dim via the interval identity (in S-units)
      S*ox = min(S*tw, S*pw, S*(tw+pw)/2 - |pc - tc|)
  inter = relu(ox)*relu(oy), union = tarea + parea - inter,
  iou_b = inter / (union + 1e-30)   # union==0 implies inter==0 -> iou 0
  sel = iou1 > iou0, selm = sel*t4, s0m = t4 - selm
  coord = 5 * sum_k s0m*(p_k-t_k)^2 + selm*(p_{5+k}-t_{5+k})^2
  conf  = s0m*(p4-iou0)^2 + selm*(p9-iou1)^2
  noobj = 0.5*(1-t4)*(p4^2+p9^2)
  class = t4 * sum_c (p_c-t_c)^2
  loss = sum(coord+conf+noobj+class)/N

Inputs are cast to bf16 on the host (tolerance 2e-2 >> bf16 error ~1e-4):
halves HBM traffic (DMA roofline ~33us/core) and enables the DVE 2x
(tensor_tensor) / 4x (tensor_scalar) bf16 perf modes. Weights fold into
Act Square scales or binary-mask tricks ((sqrt(w)*mask)^2 = w*mask);
every reduction rides Act Square+accum_out into a [128, 8] fp32 partial
the host sums. The 0/0 guard rides the Act fp32 convert as a float bias.

Layout per core: 512 samples -> 128 partitions x 4 groups, one block.
Box channels arrive as three strided DMAs (centers {0,1,5,6}, widths
{2,3,7,8}, conf {4,9}) so DVE starts at ~4us. Engines (est busy): DVE
~40us, DMA ~33us, Act ~31us, Pool ~26us. Pool gets only early-ready /
late-consumed subtractions; the per-engine queues execute in order, so
emission order is chosen to avoid cross-engine stalls.
"""

import math

import numpy as np

import concourse.mybir as mybir
from concourse import bacc
from concourse.bass_utils import run_bass_kernel_spmd
from concourse.tile import TileContext

F32 = mybir.dt.float32
BF16 = mybir.dt.bfloat16
OP = mybir.AluOpType
AF = mybir.ActivationFunctionType

N, D, S = 4096, 30, 14
SS = S * S          # 196
NCORE = 8
NPC = N // NCORE    # 512 samples per core
P = 128
G = NPC // P        # 4 groups (samples per partition)
NSLOT = 9           # coord0, coord1, conf+noobj, class x4, class-j2 halves x2

SQ5 = math.sqrt(5.0)
SQH = math.sqrt(0.5)

_CACHE = {}


def _build():
    nc = bacc.Bacc("TRN2", target_bir_lowering=False, debug=False)
    pred = nc.dram_tensor("pred", [NPC, D * SS], BF16, kind="ExternalInput")
    tgt = nc.dram_tensor("target", [NPC, D * SS], BF16, kind="ExternalInput")
    out = nc.dram_tensor("out", [P, NSLOT], F32, kind="ExternalOutput")

    pred_r = pred[:, :].rearrange("(g p) d -> p g d", g=G, p=P)
    tgt_r = tgt[:, :].rearrange("(g p) d -> p g d", g=G, p=P)
    # [P, G, box, 5*SS] strided view of the 10 box channels
    pred_b = pred_r[:, :, 0 : 10 * SS].rearrange(
        "p g (b q) -> p g b q", b=2, q=5 * SS)

    with TileContext(nc) as tc:
        with (
            tc.tile_pool(name="big", bufs=1) as big,
            tc.tile_pool(name="cls", bufs=1) as clsp,
            tc.tile_pool(name="tmp", bufs=1) as tmp,
            tc.tile_pool(name="accp", bufs=1) as accp,
        ):
            acc = accp.tile([P, NSLOT], F32)

            def slot(i):
                return acc[:, i : i + 1]

            C4 = [P, G, 2, 2, SS]
            C2 = [P, G, 2, SS]

            # ---- DMAs, in arrival-priority order ----
            pbWf = big.tile([P, G, 2, 2 * SS], BF16, tag="pbW", name="pbW")
            pbCf = big.tile([P, G, 2, 2 * SS], BF16, tag="pbC", name="pbC")
            pbF = big.tile(C2, BF16, tag="pbF", name="pbF")
            tbA = big.tile([P, G, 5, SS], BF16, tag="tbA", name="tbA")
            tbB = big.tile([P, G, 4, SS], BF16, tag="tbB", name="tbB")
            for b in range(2):
                nc.sync.dma_start(out=pbWf[:, :, b, :],
                                  in_=pred_b[:, :, b, 2 * SS : 4 * SS])
            for b in range(2):
                nc.sync.dma_start(out=pbCf[:, :, b, :],
                                  in_=pred_b[:, :, b, 0 : 2 * SS])
            pbW = pbWf[:, :, :, :].rearrange("p g b (k s) -> p g b k s",
                                             k=2, s=SS)
            pbC = pbCf[:, :, :, :].rearrange("p g b (k s) -> p g b k s",
                                             k=2, s=SS)
            nc.sync.dma_start(
                out=tbA, in_=tgt_r[:, :, 0 : 5 * SS].rearrange(
                    "p g (c s) -> p g c s", c=5, s=SS))
            nc.sync.dma_start(
                out=tbB, in_=tgt_r[:, :, 5 * SS : 9 * SS].rearrange(
                    "p g (c s) -> p g c s", c=4, s=SS))
            for b in range(2):
                nc.sync.dma_start(out=pbF[:, :, b, :],
                                  in_=pred_b[:, :, b, 4 * SS : 5 * SS])

            cls_tiles = {}
            for j in (3, 0, 1, 4, 2):   # j=1 early: its sub runs on Pool
                lo = (10 + 4 * j) * SS
                hi = lo + 4 * SS
                pc = clsp.tile([P, G, 4, SS], BF16, tag=f"pc{j}", name=f"pc{j}")
                tcl = clsp.tile([P, G, 4, SS], BF16, tag=f"tc{j}", name=f"tc{j}")
                nc.sync.dma_start(
                    out=pc, in_=pred_r[:, :, lo:hi].rearrange(
                        "p g (c s) -> p g c s", c=4, s=SS))
                nc.sync.dma_start(
                    out=tcl, in_=tgt_r[:, :, lo:hi].rearrange(
                        "p g (c s) -> p g c s", c=4, s=SS))
                cls_tiles[j] = (pc, tcl)

            tC = tbA[:, :, 0:2, :]
            tW = tbA[:, :, 2:4, :]
            t4 = tbA[:, :, 4, :]

            def bc2(x2):
                return x2.unsqueeze(2).broadcast_to((P, G, 2, 2, SS))

            def bc4(x1):
                return x1.unsqueeze(2).broadcast_to((P, G, 4, SS))

            def T(shape, tag, dt=BF16):
                return tmp.tile(shape, dt, tag=tag, name=tag)

            parea2 = T(C2, "parea2")
            tarea = T([P, G, SS], "tarea")
            s12 = T(C2, "s12")
            e4 = [T([P, G, 4, SS], "e4_0"), T([P, G, 4, SS], "e4_1")]
            wno = T([P, G, SS], "wno")
            cn = T([P, G, 4, SS], "cn")

            # ---- DVE: setup (emitted first: Pool's s12 reads these) ----
            puS = T(C4, "puS")
            pwS = T(C4, "pwS")
            nc.vector.tensor_scalar(puS, pbW, S / 2.0, None, OP.mult)
            nc.vector.tensor_scalar(pwS, pbW, float(S), None, OP.mult)
            nc.vector.tensor_mul(parea2, pbW[:, :, :, 0, :], pbW[:, :, :, 1, :])
            tuS = T(C2, "tuS")
            twS = T(C2, "twS")
            nc.vector.tensor_scalar(tuS, tW, S / 2.0, None, OP.mult)
            nc.vector.tensor_scalar(twS, tW, float(S), None, OP.mult)
            nc.vector.tensor_mul(tarea, tbA[:, :, 2, :], tbA[:, :, 3, :])
            nc.vector.tensor_scalar(wno, t4, -SQH, SQH, OP.mult, OP.add)

            # ---- Pool: early-ready, late-consumed subtractions ----
            nc.gpsimd.tensor_tensor(
                s12, parea2, tarea.unsqueeze(2).broadcast_to((P, G, 2, SS)),
                OP.add)
            nc.gpsimd.tensor_tensor(e4[0][:, :, 0:2, :], pbC[:, :, 0, :, :],
                                    tbA[:, :, 0:2, :], OP.subtract)
            nc.gpsimd.tensor_tensor(e4[0][:, :, 2:4, :], pbW[:, :, 0, :, :],
                                    tbA[:, :, 2:4, :], OP.subtract)
            nc.gpsimd.tensor_tensor(e4[1][:, :, 0:2, :], pbC[:, :, 1, :, :],
                                    tbB[:, :, 0:2, :], OP.subtract)
            nc.gpsimd.tensor_tensor(e4[1][:, :, 2:4, :], pbW[:, :, 1, :, :],
                                    tbB[:, :, 2:4, :], OP.subtract)
            nc.gpsimd.tensor_tensor(
                cn[:, :, 2:4, :], pbF,
                wno.unsqueeze(2).broadcast_to((P, G, 2, SS)), OP.mult)
            pc1, tc1 = cls_tiles[1]
            nc.gpsimd.tensor_tensor(pc1, pc1, tc1, OP.subtract)
            pc2, tc2 = cls_tiles[2]
            for half in range(2):
                hs = slice(2 * half, 2 * half + 2)
                nc.gpsimd.tensor_tensor(pc2[:, :, hs, :], pc2[:, :, hs, :],
                                        tc2[:, :, hs, :], OP.subtract)

            # ---- DVE: box chain ----
            dC = T(C4, "x4a")
            nc.vector.tensor_tensor(dC, pbC, bc2(tC), OP.subtract)
            adC = T(C4, "x4b")
            nc.scalar.activation(adC, dC, AF.Abs)          # Act
            h0 = T(C4, "x4c")
            nc.vector.tensor_tensor(h0, puS, bc2(tuS), OP.add)
            m4 = T(C4, "x4d")
            nc.vector.tensor_tensor(m4, pwS, bc2(twS), OP.min)
            h1 = T(C4, "x4a")      # reuses dC (dead after adC)
            nc.vector.tensor_sub(h1, h0, adC)
            o4 = T(C4, "x4b")      # reuses adC
            nc.vector.tensor_tensor(o4, m4, h1, OP.min)
            orr = T(C4, "x4c")     # reuses h0
            nc.vector.tensor_scalar(orr, o4, 0.0, 1.0 / S, OP.max, OP.mult)

            with tc.high_priority():
                inter2 = T(C2, "inter2")
                nc.vector.tensor_mul(
                    inter2, orr[:, :, :, 0, :], orr[:, :, :, 1, :])
                union2 = T(C2, "c2a")
                nc.vector.tensor_sub(union2, s12, inter2)
                # Act: fp32 convert with the 0/0 guard folded in as bias
                uf32 = T([P, G, 2 * SS], "uf32", dt=F32)
                nc.scalar.activation(
                    uf32.rearrange("p g (c s) -> p g c s", c=2, s=SS), union2,
                    AF.Copy, bias=1e-30)
                rf32 = T([P, G, 2 * SS], "rf32", dt=F32)
                nc.vector.reciprocal_approx_fast(out=rf32, in_=uf32)
                iou2 = T(C2, "iou2")
                nc.vector.tensor_mul(
                    iou2, inter2,
                    rf32.rearrange("p g (c s) -> p g c s", c=2, s=SS))

                # ---- masks; masks2 = [s0m, selm]; the two muls are
                # independent (s0m = sel < t4, binary) ----
                sel = T([P, G, SS], "sel")
                nc.vector.tensor_tensor(
                    sel, iou2[:, :, 1, :], iou2[:, :, 0, :], OP.is_gt)
                masks2 = T(C2, "masks2")
                nc.vector.tensor_tensor(
                    masks2[:, :, 0, :], sel, t4, OP.is_lt)
                nc.vector.tensor_mul(masks2[:, :, 1, :], sel, t4)

                # ---- conf (noobj half of cn computed on Pool above) ----
                f2 = T(C2, "f2")
                nc.vector.tensor_sub(f2, pbF, iou2)
                nc.vector.tensor_mul(cn[:, :, 0:2, :], f2, masks2)
            nc.scalar.activation(cn, cn, AF.Square, accum_out=slot(2))

            # ---- class + coord finishes, interleaved by readiness ----
            t4b = bc4(t4)

            cls_slot = {0: 3, 1: 4, 3: 5, 4: 6}

            def cls_finish(j, eng=None):
                pc, _ = cls_tiles[j]
                (eng or nc.vector).tensor_tensor(pc, pc, t4b, OP.mult)
                nc.scalar.activation(pc, pc, AF.Square,
                                     accum_out=slot(cls_slot[j]))

            def cls_sub(j):
                pc, tcl = cls_tiles[j]
                nc.vector.tensor_sub(pc, pc, tcl)

            def coord_finish(h):
                nc.vector.tensor_mul(e4[h], e4[h], bc4(masks2[:, :, h, :]))
                nc.scalar.activation(e4[h], e4[h], AF.Square, scale=SQ5,
                                     accum_out=slot(h))

            cls_sub(3)
            cls_finish(3)
            cls_sub(0)
            cls_finish(0)
            cls_finish(1)          # sub done on Pool
            cls_sub(4)
            cls_finish(4)
            coord_finish(0)
            coord_finish(1)
            # last chunk in 2-ch halves (subs on Pool): short final squares
            t4b2 = t4.unsqueeze(2).broadcast_to((P, G, 2, SS))
            for half in range(2):
                hs = slice(2 * half, 2 * half + 2)
                nc.vector.tensor_tensor(pc2[:, :, hs, :], pc2[:, :, hs, :],
                                        t4b2, OP.mult)
                nc.scalar.activation(pc2[:, :, hs, :], pc2[:, :, hs, :],
                                     AF.Square, accum_out=slot(7 + half))

            nc.sync.dma_start(out=out[:, :], in_=acc)
    nc.compile()
    return nc


def _get_nc():
    if "nc" not in _CACHE:
        _CACHE["nc"] = _build()
    return _CACHE["nc"]


def kernel(pred: np.ndarray, target: np.ndarray) -> np.ndarray:
    import ml_dtypes
    bf16 = ml_dtypes.bfloat16
    nc = _get_nc()
    p16 = np.ascontiguousarray(pred).reshape(N, D * SS).astype(bf16)
    t16 = np.ascontiguousarray(target).reshape(N, D * SS).astype(bf16)
    in_maps = []
    for k in range(NCORE):
        sl = slice(k * NPC, (k + 1) * NPC)
        in_maps.append({"pred": p16[sl], "target": t16[sl]})
    res = run_bass_kernel_spmd(nc, in_maps, core_ids=list(range(NCORE)))
    total = sum(float(r["out"].astype(np.float64).sum()) for r in res.results)
    return np.float32(total / N)
